# revision 7
# baseline (speedup 1.0000x reference)
"""MinGRU (parallel log-space scan) Trainium2 Bass kernel.

Problem (hardcoded):
    x:    [B=8, S=4096, D=1024] f32
    W_hg: [D=1024, 2*D=2048]    f32
    out:  [B=8, S=4096, D=1024] f32

    hg = x @ W_hg ; hidden, gate = split(hg)
    h_t = (1-z_t) * h_{t-1} + z_t * g(hidden_t),  z = sigmoid(gate),
    g(v) = v + 0.5 if v >= 0 else sigmoid(v)  ==  max(v + 0.5, sigmoid(v))

Sharding: data-parallel over batch, one batch row per NeuronCore (8 cores),
W_hg replicated.

Layout strategy: the scan must run along the free dimension (channels on
partitions), so the device works entirely in the transposed layout
hg^T/h^T = [channels, seq]. The host passes x pre-transposed per batch row
(converted to bf16) and transposes the returned h^T back.

bf16 matmuls: 1 cyc/row on the PE like fp32r, but FWL (fast weight load)
applies to 2-byte weight tiles so the LDWEIGHTS stream hides behind the
matmuls, and the x/W DMA volume halves. Accuracy: ~2.3e-3 max rel err
(simulated), far below the 2e-2 gate.

Per-core pipeline over seq chunks of C=512:
  DMA x^T chunk tiles [128d, C] (bf16)
  -> bf16 matmuls hg^T[k] = sum_j W[j,k]^T x^T[j] accumulated in PSUM
  -> ACT: a = sigmoid(-gate), sigh = sigmoid(hidden)      [PSUM -> SBUF]
  -> DVE: gh = (hidden + 0.5) max sigh ; bneg = (a - 1) * gh
  -> DVE: h = scan(a * h_prev) - bneg   (carry chained across chunks)
  -> DMA h^T tile straight to DRAM out^T.

Tail: the last chunk's final k-tiles run their pointwise/scan/store in
halves/quarters so the store of one piece overlaps the scan of the next.
"""

import numpy as np

import concourse.bacc as bacc
import concourse.tile as tile
from concourse import mybir

B, S, D = 8, 4096, 1024
N_CORES = 8
P = 128  # partitions
# Seq chunk schedule: small chunks first so the PE starts on ~0.5MB of
# DMA instead of ~1.3MB (the DMA engines ramp slowly for the first ~20us).
CHUNKS = [128, 384] + [512] * 7
assert sum(CHUNKS) == S
N_DT = D // P  # 8 d-tiles (contraction)
N_KT = D // P  # 8 output channel tiles (hidden dim = D)

F32 = mybir.dt.float32
BF16 = mybir.dt.bfloat16
MM_DT = BF16

_COMPILED = {}


def _build():
    nc = bacc.Bacc(
        "TRN2", target_bir_lowering=False, debug=False, num_devices=N_CORES
    )
    xt_d = nc.dram_tensor("xt", [D, S], MM_DT, kind="ExternalInput").ap()
    w_d = nc.dram_tensor("w", [D, 2 * D], MM_DT, kind="ExternalInput").ap()
    out_d = nc.dram_tensor("outT", [D, S], F32, kind="ExternalOutput").ap()

    AL = mybir.AluOpType
    SIG = mybir.ActivationFunctionType.Sigmoid

    with tile.TileContext(nc) as tc:
        with (
            tc.tile_pool(name="wpool", bufs=1) as wpool,
            tc.tile_pool(name="xtp", bufs=3) as xt_pool,
            tc.tile_pool(name="pw", bufs=3) as pw_pool,
            tc.tile_pool(name="hp", bufs=3) as h_pool,
            tc.tile_pool(name="pshg", bufs=8, space="PSUM") as psum_hg,
        ):
            def load_x_chunk(s0, csz, names):
                tiles = []
                for j in range(N_DT):
                    t = xt_pool.tile(
                        [P, csz],
                        MM_DT,
                        tag=f"xt{j}",
                        name=names[j] if names else None,
                    )
                    nc.sync.dma_start(
                        t[:], xt_d[j * P : (j + 1) * P, s0 : s0 + csz]
                    )
                    tiles.append(t)
                return tiles

            w_big = [
                wpool.tile([P, 2 * D], MM_DT, tag=f"w{j}", name=f"w_big{j}")
                for j in range(N_DT)
            ]

            def wload(k0, k1):
                # one DMA per j: 3D AP covering the hidden cols [k0*P, k1*P)
                # and the matching gate cols D + [k0*P, k1*P) together
                for j in range(N_DT):
                    dst = w_big[j].rearrange("p (b c) -> p b c", b=2)
                    src = w_d[j * P : (j + 1) * P, :].rearrange(
                        "r (b c) -> r b c", b=2
                    )
                    nc.sync.dma_start(
                        dst[:, :, k0 * P : k1 * P], src[:, :, k0 * P : k1 * P]
                    )

            # Critical path first: the k=0 weight slice (both hidden and
            # gate halves) and the small chunk 0 of x^T, so the PE starts
            # on ~0.5MB of DMA.
            wload(0, 1)
            x0 = load_x_chunk(0, CHUNKS[0], [f"x0_{j}" for j in range(N_DT)])
            wload(1, 4)
            x1 = load_x_chunk(
                CHUNKS[0], CHUNKS[1], [f"x1_{j}" for j in range(N_DT)]
            )
            wload(4, 8)
            w_sb = [
                [w_big[j][:, kk * P : (kk + 1) * P] for j in range(N_DT)]
                for kk in range(2 * N_KT)
            ]

            def mm_group(ps, kk, xts, lo, hi):
                for j in range(N_DT):
                    nc.tensor.matmul(
                        ps[:, lo:hi],
                        w_sb[kk][j],
                        xts[j][:, lo:hi],
                        start=(j == 0),
                        stop=(j == N_DT - 1),
                    )

            prev_h = [None] * N_KT
            s0 = 0
            for sc, csz in enumerate(CHUNKS):
                last_chunk = sc == len(CHUNKS) - 1
                # ---- load x^T chunk tiles [128d, csz]
                if sc == 0:
                    xts = x0
                elif sc == 1:
                    xts = x1
                else:
                    xts = load_x_chunk(s0, csz, None)
                # ---- per channel-tile k: matmuls + pointwise + scan + store
                for k in range(N_KT):
                    last_k = last_chunk and k == N_KT - 1
                    # gate first: a = sigmoid(-gate) is ready while the
                    # hidden matmuls run, shortening the per-k tail chain
                    pg = psum_hg.tile([P, csz], F32, tag="ph")  # gate
                    mm_group(pg, N_KT + k, xts, 0, csz)
                    a_t = pw_pool.tile([P, csz], F32, tag="a")
                    nc.scalar.activation(a_t[:], pg[:], SIG, scale=-1.0)
                    if last_k:
                        # split the last accumulation (separate PSUM banks:
                        # a start flag zeroes the whole 2KB zero-region) so
                        # the pointwise tail starts before the final matmul
                        hhalf = csz // 2
                        ph_a = psum_hg.tile([P, hhalf], F32, tag="ph")
                        ph_b = psum_hg.tile([P, hhalf], F32, tag="ph")
                        for j in range(N_DT):
                            nc.tensor.matmul(
                                ph_a[:], w_sb[k][j], xts[j][:, 0:hhalf],
                                start=(j == 0), stop=(j == N_DT - 1),
                            )
                        for j in range(N_DT):
                            nc.tensor.matmul(
                                ph_b[:], w_sb[k][j], xts[j][:, hhalf:csz],
                                start=(j == 0), stop=(j == N_DT - 1),
                            )

                        def ph_piece(lo, hi):
                            if hi <= hhalf:
                                return ph_a[:, lo:hi]
                            assert lo >= hhalf
                            return ph_b[:, lo - hhalf : hi - hhalf]
                    else:
                        ph = psum_hg.tile([P, csz], F32, tag="ph")  # hidden
                        mm_group(ph, k, xts, 0, csz)

                        def ph_piece(lo, hi):
                            return ph[:, lo:hi]
                    # pointwise/scan splits: finer at the kernel tail so the
                    # last stores overlap the last scans
                    if last_k:
                        nsplit = 4
                    elif last_chunk and k >= N_KT - 3:
                        nsplit = 2
                    else:
                        nsplit = 1
                    sigh = pw_pool.tile([P, csz], F32, tag="sigh")
                    gh = pw_pool.tile([P, csz], F32, tag="gh")
                    bneg = pw_pool.tile([P, csz], F32, tag="bneg")
                    h = h_pool.tile([P, csz], F32, tag=f"h{k}")
                    H = csz // nsplit
                    for q in range(nsplit):
                        lo, hi = q * H, (q + 1) * H
                        php = ph_piece(lo, hi)
                        # sigh = sigmoid(hidden)
                        nc.scalar.activation(sigh[:, lo:hi], php, SIG)
                        # g(hidden) = max(hidden + 0.5, sigmoid(hidden))
                        nc.vector.scalar_tensor_tensor(
                            gh[:, lo:hi], php, 0.5, sigh[:, lo:hi],
                            op0=AL.add, op1=AL.max,
                        )
                        # bneg = (a - 1) * g = -(z * g)
                        nc.vector.scalar_tensor_tensor(
                            bneg[:, lo:hi], a_t[:, lo:hi], 1.0, gh[:, lo:hi],
                            op0=AL.subtract, op1=AL.mult,
                        )
                        # h_t = a_t * h_{t-1} - bneg_t  (linear recurrence)
                        if q == 0:
                            init = (
                                0.0
                                if prev_h[k] is None
                                else prev_h[k][:, -1:]
                            )
                        else:
                            init = h[:, lo - 1 : lo]
                        nc.vector.tensor_tensor_scan(
                            h[:, lo:hi], a_t[:, lo:hi], bneg[:, lo:hi], init,
                            op0=AL.mult, op1=AL.subtract,
                        )
                        nc.sync.dma_start(
                            out_d[k * P : (k + 1) * P, s0 + lo : s0 + hi],
                            h[:, lo:hi],
                        )
                    prev_h[k] = h
                s0 += csz
    nc.compile()
    return nc


def _get_nc():
    key = str(MM_DT)
    if key not in _COMPILED:
        _COMPILED[key] = _build()
    return _COMPILED[key]


def make_in_maps(x: np.ndarray, W_hg: np.ndarray) -> list[dict]:
    import ml_dtypes

    bf = ml_dtypes.bfloat16
    x = np.asarray(x, dtype=np.float32)
    w = np.ascontiguousarray(np.asarray(W_hg, dtype=np.float32).astype(bf))
    return [
        {"xt": np.ascontiguousarray(x[b].T.astype(bf)), "w": w}
        for b in range(N_CORES)
    ]


def kernel(x: np.ndarray, W_hg: np.ndarray) -> np.ndarray:
    from concourse.bass_utils import run_bass_kernel_spmd

    assert x.shape == (B, S, D) and W_hg.shape == (D, 2 * D)
    nc = _get_nc()
    in_maps = make_in_maps(x, W_hg)
    res = run_bass_kernel_spmd(nc, in_maps, list(range(N_CORES)))
    out = np.empty((B, S, D), dtype=np.float32)
    for b in range(N_CORES):
        out[b] = res.results[b]["outT"].T
    return out


# revision 9
# speedup vs baseline: 1.0689x; 1.0689x over previous
"""MinGRU (parallel log-space scan) Trainium2 Bass kernel.

Problem (hardcoded):
    x:    [B=8, S=4096, D=1024] f32
    W_hg: [D=1024, 2*D=2048]    f32
    out:  [B=8, S=4096, D=1024] f32

    hg = x @ W_hg ; hidden, gate = split(hg)
    h_t = (1-z_t) * h_{t-1} + z_t * g(hidden_t),  z = sigmoid(gate),
    g(v) = v + 0.5 if v >= 0 else sigmoid(v)  ==  max(v + 0.5, sigmoid(v))

Sharding: data-parallel over batch, one batch row per NeuronCore (8 cores),
W_hg replicated.

Layout strategy: the scan must run along the free dimension (channels on
partitions), so the device works entirely in the transposed layout
hg^T/h^T = [channels, seq]. The host packs x per batch row into
per-chunk-contiguous bf16 blocks and W into per-k-slice-contiguous bf16
blocks so every SBUF load is a single DMA instruction (the Sync engine
serializes DMA issues at ~0.6us each, which otherwise dominates the
kernel head).

bf16 matmuls: 1 cyc/row on the PE like fp32r, but FWL (fast weight load)
hides the LDWEIGHTS stream behind the matmuls, and the x/W DMA volume
halves. Accuracy: ~2.3e-3 max rel err, far below the 2e-2 gate.

Per-core pipeline over seq chunks of C=512:
  one DMA for the x^T chunk block [128, 8j x C] (bf16)
  -> per k: bf16 matmuls gate then hidden, accumulated in PSUM
     (a = sigmoid(-gate) on ACT overlaps the hidden matmuls)
  -> DVE: gh = (hidden + 0.5) max sigh ; bneg = (a - 1) * gh
  -> DVE: h = scan(a * h_prev) - bneg   (carry chained across chunks)
  -> DMA h^T tile straight to DRAM out^T.

Tail: the last k-tile's hidden accumulation is split in half and its
pointwise/scan/store runs in quarters so the final stores overlap the
final scans.
"""

import numpy as np

import concourse.bacc as bacc
import concourse.tile as tile
from concourse import mybir

B, S, D = 8, 4096, 1024
N_CORES = 8
P = 128  # partitions
C = 512  # seq chunk
N_CHUNKS = S // C  # 8
N_DT = D // P  # 8 d-tiles (contraction)
N_KT = D // P  # 8 output channel tiles (hidden dim = D)
XBLK = N_DT * C  # packed x chunk block columns
WBLK = N_DT * 2 * P  # packed w k-slice columns (j-major, hidden+gate)

F32 = mybir.dt.float32
BF16 = mybir.dt.bfloat16
MM_DT = BF16

_COMPILED = {}


def _build():
    nc = bacc.Bacc(
        "TRN2", target_bir_lowering=False, debug=False, num_devices=N_CORES
    )
    # packed layouts (see make_in_maps): one contiguous run per SBUF load
    xt_d = nc.dram_tensor(
        "xt", [P, N_CHUNKS * XBLK], MM_DT, kind="ExternalInput"
    ).ap()
    w_d = nc.dram_tensor(
        "w", [P, N_KT * WBLK], MM_DT, kind="ExternalInput"
    ).ap()
    out_d = nc.dram_tensor("outT", [D, S], F32, kind="ExternalOutput").ap()

    AL = mybir.AluOpType
    SIG = mybir.ActivationFunctionType.Sigmoid

    with tile.TileContext(nc) as tc:
        with (
            tc.tile_pool(name="wpool", bufs=1) as wpool,
            tc.tile_pool(name="xtp", bufs=3) as xt_pool,
            tc.tile_pool(name="pw", bufs=3) as pw_pool,
            tc.tile_pool(name="hp", bufs=3) as h_pool,
            tc.tile_pool(name="pshg", bufs=8, space="PSUM") as psum_hg,
        ):
            w_tile = wpool.tile([P, N_KT * WBLK], MM_DT, name="w_tile")

            def wload(k):
                nc.sync.dma_start(
                    w_tile[:, k * WBLK : (k + 1) * WBLK],
                    w_d[:, k * WBLK : (k + 1) * WBLK],
                )

            def load_x_chunk(sc, name):
                t = xt_pool.tile([P, XBLK], MM_DT, tag="xc", name=name)
                nc.sync.dma_start(
                    t[:], xt_d[:, sc * XBLK : (sc + 1) * XBLK]
                )
                return t

            # Critical path first: k=0 weight slice then chunk 0 of x^T,
            # each a single DMA issue.
            wload(0)
            x0 = load_x_chunk(0, "x0")
            wload(1)
            x1 = load_x_chunk(1, "x1")
            for k in range(2, N_KT):
                wload(k)

            # lhsT slices: w_sb[kk][j]; kk in [0,8) hidden, [8,16) gate
            w_sb = [
                [
                    w_tile[
                        :,
                        k * WBLK + j * 2 * P + b * P :
                        k * WBLK + j * 2 * P + (b + 1) * P,
                    ]
                    for j in range(N_DT)
                ]
                for b in range(2)
                for k in range(N_KT)
            ]

            prev_h = [None] * N_KT
            for sc in range(N_CHUNKS):
                s0 = sc * C
                last_chunk = sc == N_CHUNKS - 1
                if sc == 0:
                    xts = x0
                elif sc == 1:
                    xts = x1
                else:
                    xts = load_x_chunk(sc, None)

                def mm_group(ps, kk, lo, hi):
                    for j in range(N_DT):
                        nc.tensor.matmul(
                            ps[:],
                            w_sb[kk][j],
                            xts[:, j * C + lo : j * C + hi],
                            start=(j == 0),
                            stop=(j == N_DT - 1),
                        )

                for k in range(N_KT):
                    last_k = last_chunk and k == N_KT - 1
                    # gate first: a = sigmoid(-gate) is ready while the
                    # hidden matmuls run, shortening the per-k tail chain
                    pg = psum_hg.tile([P, C], F32, tag="ph")  # gate
                    mm_group(pg, N_KT + k, 0, C)
                    a_t = pw_pool.tile([P, C], F32, tag="a")
                    nc.scalar.activation(a_t[:], pg[:], SIG, scale=-1.0)
                    if last_k:
                        # split the last accumulation (separate PSUM banks:
                        # a start flag zeroes the whole 2KB zero-region) so
                        # the pointwise tail starts before the final matmul
                        hh = C // 2
                        ph_a = psum_hg.tile([P, hh], F32, tag="ph")
                        ph_b = psum_hg.tile([P, hh], F32, tag="ph")
                        for j in range(N_DT):
                            nc.tensor.matmul(
                                ph_a[:], w_sb[k][j],
                                xts[:, j * C : j * C + hh],
                                start=(j == 0), stop=(j == N_DT - 1),
                            )
                        for j in range(N_DT):
                            nc.tensor.matmul(
                                ph_b[:], w_sb[k][j],
                                xts[:, j * C + hh : (j + 1) * C],
                                start=(j == 0), stop=(j == N_DT - 1),
                            )

                        def ph_piece(lo, hi):
                            if hi <= hh:
                                return ph_a[:, lo:hi]
                            assert lo >= hh
                            return ph_b[:, lo - hh : hi - hh]
                    else:
                        ph = psum_hg.tile([P, C], F32, tag="ph")  # hidden
                        mm_group(ph, k, 0, C)

                        def ph_piece(lo, hi):
                            return ph[:, lo:hi]

                    # pointwise/scan splits: finer at the kernel tail so
                    # the last stores overlap the last scans
                    if last_k:
                        nsplit = 4
                    elif last_chunk and k >= N_KT - 3:
                        nsplit = 2
                    else:
                        nsplit = 1
                    sigh = pw_pool.tile([P, C], F32, tag="sigh")
                    gh = pw_pool.tile([P, C], F32, tag="gh")
                    bneg = pw_pool.tile([P, C], F32, tag="bneg")
                    h = h_pool.tile([P, C], F32, tag=f"h{k}")
                    H = C // nsplit
                    for q in range(nsplit):
                        lo, hi = q * H, (q + 1) * H
                        php = ph_piece(lo, hi)
                        # sigh = sigmoid(hidden)
                        nc.scalar.activation(sigh[:, lo:hi], php, SIG)
                        # g(hidden) = max(hidden + 0.5, sigmoid(hidden))
                        nc.vector.scalar_tensor_tensor(
                            gh[:, lo:hi], php, 0.5, sigh[:, lo:hi],
                            op0=AL.add, op1=AL.max,
                        )
                        # bneg = (a - 1) * g = -(z * g)
                        nc.vector.scalar_tensor_tensor(
                            bneg[:, lo:hi], a_t[:, lo:hi], 1.0, gh[:, lo:hi],
                            op0=AL.subtract, op1=AL.mult,
                        )
                        # h_t = a_t * h_{t-1} - bneg_t  (linear recurrence)
                        if q == 0:
                            init = (
                                0.0
                                if prev_h[k] is None
                                else prev_h[k][:, -1:]
                            )
                        else:
                            init = h[:, lo - 1 : lo]
                        nc.vector.tensor_tensor_scan(
                            h[:, lo:hi], a_t[:, lo:hi], bneg[:, lo:hi], init,
                            op0=AL.mult, op1=AL.subtract,
                        )
                        nc.sync.dma_start(
                            out_d[k * P : (k + 1) * P, s0 + lo : s0 + hi],
                            h[:, lo:hi],
                        )
                    prev_h[k] = h
    nc.compile()
    return nc


def _get_nc():
    key = str(MM_DT)
    if key not in _COMPILED:
        _COMPILED[key] = _build()
    return _COMPILED[key]


def make_in_maps(x: np.ndarray, W_hg: np.ndarray) -> list[dict]:
    import ml_dtypes

    bf = ml_dtypes.bfloat16
    x = np.asarray(x, dtype=np.float32)
    w = np.asarray(W_hg, dtype=np.float32)
    # x pack: [sc, t, j, p] -> [p, sc, j, t] (per-chunk contiguous blocks)
    xp = [
        np.ascontiguousarray(
            x[b]
            .reshape(N_CHUNKS, C, N_DT, P)
            .transpose(3, 0, 2, 1)
            .reshape(P, N_CHUNKS * XBLK)
            .astype(bf)
        )
        for b in range(N_CORES)
    ]
    # w pack: W[j*128+p, b*1024 + k*128 + c] -> wp[p, k, j, b, c]
    wp = np.ascontiguousarray(
        w.reshape(N_DT, P, 2, N_KT, P)
        .transpose(1, 3, 0, 2, 4)
        .reshape(P, N_KT * WBLK)
        .astype(bf)
    )
    return [{"xt": xp[b], "w": wp} for b in range(N_CORES)]


def kernel(x: np.ndarray, W_hg: np.ndarray) -> np.ndarray:
    from concourse.bass_utils import run_bass_kernel_spmd

    assert x.shape == (B, S, D) and W_hg.shape == (D, 2 * D)
    nc = _get_nc()
    in_maps = make_in_maps(x, W_hg)
    res = run_bass_kernel_spmd(nc, in_maps, list(range(N_CORES)))
    out = np.empty((B, S, D), dtype=np.float32)
    for b in range(N_CORES):
        out[b] = res.results[b]["outT"].T
    return out
